# revision 3
# baseline (speedup 1.0000x reference)
"""MultiHeadAttention Trainium2 kernel (8 NeuronCores, data-parallel over batch).

Problem: B=8, S=1024, D=1024, E=1024, H=16 heads, Dh=64.
  qkv = x @ qkv_w.T + qkv_b ; per head: softmax(q k^T) @ v ; out = vals @ o_w.T + o_b
  (softmax on UNSCALED logits, faithful to the reference.)

Strategy (v8 — fused region, minimal instruction pressure)
----------------------------------------------------------
- Data-parallel: core b processes batch element b completely. No collectives.
- Mixed precision validated vs the fp64 reference (rel err ~3.2e-3, gate 2e-2):
  fp16 for x / qkv(q,k) weights / q / k (logits path needs the mantissa),
  bf16 for v / exp(logits) / normalized vals / o_w (exp needs bf16 range).
- The engine SEQUENCERS (PE/SP) are near-saturated alongside the PE array, so
  v8 minimizes instruction count everywhere:
    * every projection (q/k, v, o) accumulates a [128,1024] PSUM tile
      (two one-bank halves) -> ONE evac instruction per group;
    * q and k weights for a head pair arrive in ONE DMA;
    * softmax normalization for ALL pairs uses a K=1 ones-stationary matmul
      to broadcast 1/rowsum across partitions (kills 48 serialized
      SP-sequencer DMA dispatches of the DRAM-bounce approach);
    * o-proj evacuates on the otherwise-idle ACT engine at [128,1024].
- PSUM budget exactly 8 banks: pl ring 2x[128,1024] (logits AND all
  projection groups; double-buffered so the ACT exp chain pipelines),
  pav 2x[65,512] (attn@v accumulators, query-half split, c-outer),
  proj ring 2x[128,512] reserved for the reciprocal broadcasts.
- attn@v rides a ones column producing the softmax denominator in row 64;
  fast PSUM evac frees the accumulator, normalize runs off-critical-path.
- o_b and the v-bias contribution are folded in on the host (softmax rows
  sum to 1).
"""

import numpy as np
import ml_dtypes

import concourse.bass as bass
import concourse.tile as tile
from concourse import bacc, mybir
from concourse.bass_utils import run_bass_kernel_spmd

F32 = mybir.dt.float32
F32R = mybir.dt.float32r
F16 = mybir.dt.float16
BF16 = mybir.dt.bfloat16
EXP = mybir.ActivationFunctionType.Exp

B, S, D, E, H, Dh = 8, 1024, 1024, 1024, 16, 64
P = 128          # partitions
NT = S // P      # 8 s-tiles
ND = D // P      # 8 d-tiles
NPAIR = H // 2   # 8 head-pair tiles
FD = 512         # matmul moving free dim

N_CORES = 8


def build_nc(reps: int = 1):
    nc = bacc.Bacc("TRN2", target_bir_lowering=False, debug=False,
                   num_devices=N_CORES)

    xT_d = nc.declare_dram_parameter("xT", [D, S], F16, isOutput=False)
    wqk_d = nc.declare_dram_parameter("wqk", [NPAIR, P, 2, ND, P], F16,
                                      isOutput=False)
    wvT_d = nc.declare_dram_parameter("wvT", [2, P, ND, FD], F16,
                                      isOutput=False)
    owT_d = nc.declare_dram_parameter("owT", [P, NPAIR, E], BF16,
                                      isOutput=False)
    bqk_d = nc.declare_dram_parameter("bqk", [P, 2 * NPAIR], F32,
                                      isOutput=False)
    out_d = nc.declare_dram_parameter("out", [S, E], F32, isOutput=True)

    with tile.TileContext(nc) as tc:
      for _rep in range(reps):
        with (
            tc.tile_pool(name="glob", bufs=1) as glob,
            tc.tile_pool(name="wpool", bufs=1) as wpool,
            tc.tile_pool(name="pwv", bufs=2) as pwv,
            tc.tile_pool(name="pwqk", bufs=3) as pwqk,
            tc.tile_pool(name="pexp", bufs=28) as pexp,
            tc.tile_pool(name="pnrm", bufs=3) as pnrm,
            tc.tile_pool(name="pout", bufs=2) as pout,
            tc.tile_pool(name="psl", bufs=2, space="PSUM") as psl,
            tc.tile_pool(name="psrc", bufs=2, space="PSUM") as psrc,
            tc.tile_pool(name="psav", bufs=2, space="PSUM") as psav,
        ):
            # ---------------- global tiles + DMAs ----------------
            bqk_sb = glob.tile([P, 2 * NPAIR], F32)
            nc.sync.dma_start(bqk_sb[:], bqk_d[:])

            xT_sb = wpool.tile([P, ND, S], F16)
            qT_sb = wpool.tile([P, NPAIR, S], F16)   # [64p+j, pair, s]
            kT_sb = wpool.tile([P, NPAIR, S], F16)
            v_sb = wpool.tile([P, NT, H, Dh + 1], BF16)
            valsN = wpool.tile([P, NPAIR, S], BF16)  # head-pair packed vals^T
            owT_sb = wpool.tile([P, NPAIR, E], BF16)

            # DMA order tuned so qk0's first matmul starts early
            xT_r = xT_d.rearrange("(dt p) s -> p dt s", p=P)
            wqk_t = {}
            w_t = pwqk.tile([P, 2, ND, P], F16, tag="wqk", name="w_t")
            nc.sync.dma_start(w_t[:], wqk_d[0])
            wqk_t[0] = w_t
            nc.sync.dma_start(xT_sb[:, :, 0:FD], xT_r[:, :, 0:FD])
            w_t = pwqk.tile([P, 2, ND, P], F16, tag="wqk", name="w_t")
            nc.sync.dma_start(w_t[:], wqk_d[1])
            wqk_t[1] = w_t
            nc.sync.dma_start(xT_sb[:, :, FD:S], xT_r[:, :, FD:S])
            wv_c = []
            for c in range(2):
                wv = pwv.tile([P, ND, FD], F16, tag="wv", name="wv")
                nc.sync.dma_start(wv[:], wvT_d[c])
                wv_c.append(wv)
            nc.sync.dma_start(owT_sb[:], owT_d[:])

            # ones column of the augmented v + ones stationary for the K=1
            # reciprocal-broadcast matmul (memset can't write f32r/bf16)
            ones_t = glob.tile([P, 1], F32)
            nc.vector.memset(ones_t[:], 1.0)
            nc.vector.tensor_copy(
                out=v_sb[:, :, :, Dh:Dh + 1],
                in_=ones_t[:, None, None, :].to_broadcast((P, NT, H, 1)))
            oc_f = glob.tile([1, P], F32)
            nc.vector.memset(oc_f[:], 1.0)
            oc = glob.tile([1, P], F32R)
            nc.vector.tensor_copy(out=oc[:], in_=oc_f[:])

            def qk_proj(t):
                """q/k projections for head pair t (own psum ring so they
                prefetch during the previous pair's exp chain)."""
                w_t = wqk_t.pop(t, None)
                if w_t is None:
                    w_t = pwqk.tile([P, 2, ND, P], F16, tag="wqk",
                                    name="w_t")
                    nc.sync.dma_start(w_t[:], wqk_d[t])
                for qk in range(2):
                    dest = qT_sb if qk == 0 else kT_sb
                    for c in range(2):
                        ps = psrc.tile([P, FD], F32, tag="rcp", name="ps")
                        for dt in range(ND):
                            nc.tensor.matmul(
                                ps[:],
                                w_t[:, qk, dt, :],
                                xT_sb[:, dt, FD * c:FD * (c + 1)],
                                start=(dt == 0), stop=(dt == ND - 1))
                        nc.vector.tensor_scalar(
                            out=dest[:, t, FD * c:FD * (c + 1)],
                            in0=ps[:],
                            scalar1=bqk_sb[:,
                                           NPAIR * qk + t:NPAIR * qk + t + 1],
                            scalar2=None,
                            op0=mybir.AluOpType.add)

            qk_proj(0)

            # v projection: one [128,1024] group per s-tile (both c halves)
            for st in range(NT):
                ps = psl.tile([P, S], F32, tag="pl", name="ps")
                for dt in range(ND):
                    for c in range(2):
                        nc.tensor.matmul(
                            ps[:, FD * c:FD * (c + 1)],
                            xT_sb[:, dt, P * st:P * (st + 1)],
                            wv_c[c][:, dt, :],
                            start=(dt == 0), stop=(dt == ND - 1))
                nc.vector.tensor_copy(
                    out=v_sb[:, st, :, 0:Dh],
                    in_=ps[:].rearrange("p (c h e) -> p (c h) e", c=2, h=8))

            # ---------------- fused attention pair loop ----------------
            for t in range(NPAIR):
                # logits + exp, st-major. The two heads of the pair live on
                # PE row-groups 0-1 (partitions 0:64) and 2-3 (64:128); with
                # p-interleaved issue the two K=64 matmuls run CONCURRENTLY
                # (tile_position auto-derives from the operand base
                # partition), doubling logits throughput vs serial issue.
                ex_t = [[None, None] for _ in range(NT)]
                for st in range(NT):
                    pl_p = [psl.tile([P, S], F32, tag="pl", name="pl")
                            for _ in range(2)]
                    for c in range(2):
                        for p in range(2):
                            b0 = Dh * p
                            nc.tensor.matmul(
                                pl_p[p][:, FD * c:FD * (c + 1)],
                                kT_sb[b0:b0 + Dh, t, P * st:P * (st + 1)],
                                qT_sb[b0:b0 + Dh, t, FD * c:FD * (c + 1)],
                                start=True, stop=True)
                    for p in range(2):
                        ex = pexp.tile([P, S], BF16, tag="ex", name="ex")
                        nc.scalar.activation(ex[:], pl_p[p][:], EXP)
                        ex_t[st][p] = ex

                # next pair's q/k projections fill PE gaps in the exp chain
                if t + 1 < NPAIR:
                    qk_proj(t + 1)

                # attn@v, query-half (c) outer so pav fits 2 banks
                for c in range(2):
                    pav = [psav.tile([Dh + 1, FD], F32, tag="pav",
                                     name="pav") for _ in range(2)]
                    for st in range(NT):
                        for p in range(2):
                            nc.tensor.matmul(
                                pav[p][:],
                                v_sb[:, st, 2 * t + p, :],
                                ex_t[st][p][:, FD * c:FD * (c + 1)],
                                start=(st == 0), stop=(st == NT - 1))
                    cs = slice(FD * c, FD * (c + 1))
                    for p in range(2):
                        # fast PSUM evac (frees pav), then normalize: DVE
                        # reciprocal of the row-64 denominator, K=1 matmul
                        # partition-broadcast, multiply into bf16 valsN
                        vU = pnrm.tile([Dh + 1, FD], F32, tag="vU",
                                       name="vU")
                        nc.vector.tensor_copy(out=vU[:], in_=pav[p][:])
                        rc = pnrm.tile([1, FD], F32R, tag="rc", name="rc")
                        # f32r is bit-identical fp32 (PE addressing tag)
                        with nc.allow_low_precision(reason="f32r==f32"):
                            nc.vector.reciprocal(rc[:], vU[Dh:Dh + 1, :])
                        rcp = psrc.tile([P, FD], F32, tag="rcp", name="rcp")
                        nc.tensor.matmul(rcp[:], oc[:], rc[:],
                                         start=True, stop=True)
                        if p == 0:
                            nc.vector.tensor_mul(
                                valsN[0:Dh, t, cs], vU[0:Dh, :],
                                rcp[0:Dh, :])
                        else:
                            tmp = pnrm.tile([Dh, FD], BF16, tag="vtmp",
                                            name="tmp")
                            nc.vector.tensor_mul(
                                tmp[:], vU[0:Dh, :], rcp[0:Dh, :])
                            nc.sync.dma_start(valsN[Dh:P, t, cs], tmp[:])

            # ---------------- output projection ----------------
            for m in range(NT):
                ps = psl.tile([P, S], F32, tag="pl", name="ps")
                for tt in range(NPAIR):
                    for c in range(2):
                        nc.tensor.matmul(
                            ps[:, FD * c:FD * (c + 1)],
                            valsN[:, tt, P * m:P * (m + 1)],
                            owT_sb[:, tt, FD * c:FD * (c + 1)],
                            start=(tt == 0), stop=(tt == NPAIR - 1))
                ot = pout.tile([P, S], F32, tag="ot", name="ot")
                nc.scalar.copy(ot[:], ps[:])
                nc.sync.dma_start(out_d[P * m:P * (m + 1), :], ot[:])

    nc.compile()
    return nc


_NC_CACHE = {}


def get_nc():
    if "nc" not in _NC_CACHE:
        _NC_CACHE["nc"] = build_nc()
    return _NC_CACHE["nc"]


def prepare_inputs(x, qkv_w, qkv_b, o_w, o_b):
    """Host-side layout packing. Returns (in_maps, correction)."""
    x = np.asarray(x, dtype=np.float32)
    qkv_w = np.asarray(qkv_w, dtype=np.float32)
    qkv_b = np.asarray(qkv_b, dtype=np.float32)
    o_w = np.asarray(o_w, dtype=np.float32)
    o_b = np.asarray(o_b, dtype=np.float32)

    w3 = qkv_w.reshape(H, 3 * Dh, D)
    wq = w3[:, 0:Dh, :].reshape(E, D)        # row 64h+j = q_j of head h
    wk = w3[:, Dh:2 * Dh, :].reshape(E, D)
    wv = w3[:, 2 * Dh:, :].reshape(E, D)

    wqk = np.concatenate([wq, wk], axis=0)   # [2048, 1024]
    wqkT = np.ascontiguousarray(wqk.T)       # [D, 2048]
    # [2*NPAIR, P, ND, P] with tile index t (q) / t+NPAIR (k) ...
    wqk_tiles = wqkT.reshape(ND, P, 2 * NPAIR, P).transpose(2, 1, 0, 3)
    # ... re-packed so pair t = [q_t, k_t] arrives in one DMA
    wqk_pair = np.stack(
        [np.stack([wqk_tiles[t], wqk_tiles[t + NPAIR]], axis=1)
         for t in range(NPAIR)], axis=0)     # [NPAIR, P, 2, ND, P]
    wqk_pair = np.ascontiguousarray(wqk_pair).astype(np.float16)

    wvT = np.ascontiguousarray(wv.T)         # [D, E]
    wvT_tiled = np.ascontiguousarray(
        wvT.reshape(ND, P, 2, FD).transpose(2, 1, 0, 3)).astype(np.float16)

    owT = np.ascontiguousarray(o_w.T)        # [E, E]; row e = 128t + r
    owT_pair = np.ascontiguousarray(
        owT.reshape(NPAIR, P, E).transpose(1, 0, 2)).astype(ml_dtypes.bfloat16)

    b3 = qkv_b.reshape(H, 3 * Dh)
    bq, bk, bv = b3[:, 0:Dh], b3[:, Dh:2 * Dh], b3[:, 2 * Dh:]
    cols = [np.concatenate([bq[2 * t], bq[2 * t + 1]]) for t in range(NPAIR)]
    cols += [np.concatenate([bk[2 * t], bk[2 * t + 1]]) for t in range(NPAIR)]
    bqk = np.ascontiguousarray(np.stack(cols, axis=1))  # [128, 16]

    correction = bv.reshape(E) @ o_w.T + o_b            # [E]

    in_maps = []
    for b in range(B):
        in_maps.append({
            "xT": np.ascontiguousarray(x[b].T).astype(np.float16),
            "wqk": wqk_pair,
            "wvT": wvT_tiled,
            "owT": owT_pair,
            "bqk": bqk,
        })
    return in_maps, correction


def kernel(x, qkv_w, qkv_b, o_w, o_b):
    nc = get_nc()
    in_maps, correction = prepare_inputs(x, qkv_w, qkv_b, o_w, o_b)
    res = run_bass_kernel_spmd(nc, in_maps, list(range(N_CORES)))
    out = np.stack([res.results[b]["out"] for b in range(B)], axis=0)
    out = out + correction[None, None, :]
    return out.astype(np.float32)



# revision 8
# speedup vs baseline: 1.0193x; 1.0193x over previous
"""MultiHeadAttention Trainium2 kernel (8 NeuronCores, data-parallel over batch).

Problem: B=8, S=1024, D=1024, E=1024, H=16 heads, Dh=64.
  qkv = x @ qkv_w.T + qkv_b ; per head: softmax(q k^T) @ v ; out = vals @ o_w.T + o_b
  (softmax on UNSCALED logits, faithful to the reference.)

Strategy (v8 — fused region, minimal instruction pressure)
----------------------------------------------------------
- Data-parallel: core b processes batch element b completely. No collectives.
- Mixed precision validated vs the fp64 reference (rel err ~3.2e-3, gate 2e-2):
  fp16 for x / qkv(q,k) weights / q / k (logits path needs the mantissa),
  bf16 for v / exp(logits) / normalized vals / o_w (exp needs bf16 range).
- The engine SEQUENCERS (PE/SP) are near-saturated alongside the PE array, so
  v8 minimizes instruction count everywhere:
    * every projection (q/k, v, o) accumulates a [128,1024] PSUM tile
      (two one-bank halves) -> ONE evac instruction per group;
    * q and k weights for a head pair arrive in ONE DMA;
    * softmax normalization for ALL pairs uses a K=1 ones-stationary matmul
      to broadcast 1/rowsum across partitions (kills 48 serialized
      SP-sequencer DMA dispatches of the DRAM-bounce approach);
    * o-proj evacuates on the otherwise-idle ACT engine at [128,1024].
- PSUM budget exactly 8 banks: pl ring 2x[128,1024] (logits AND all
  projection groups; double-buffered so the ACT exp chain pipelines),
  pav 2x[65,512] (attn@v accumulators, query-half split, c-outer),
  proj ring 2x[128,512] reserved for the reciprocal broadcasts.
- attn@v rides a ones column producing the softmax denominator in row 64;
  fast PSUM evac frees the accumulator, normalize runs off-critical-path.
- o_b and the v-bias contribution are folded in on the host (softmax rows
  sum to 1).
"""

import numpy as np
import ml_dtypes

import concourse.bass as bass
import concourse.tile as tile
from concourse import bacc, mybir
from concourse.bass_utils import run_bass_kernel_spmd

F32 = mybir.dt.float32
F32R = mybir.dt.float32r
F16 = mybir.dt.float16
BF16 = mybir.dt.bfloat16
EXP = mybir.ActivationFunctionType.Exp

B, S, D, E, H, Dh = 8, 1024, 1024, 1024, 16, 64
P = 128          # partitions
NT = S // P      # 8 s-tiles
ND = D // P      # 8 d-tiles
NPAIR = H // 2   # 8 head-pair tiles
FD = 512         # matmul moving free dim

N_CORES = 8


def build_nc(reps: int = 1):
    nc = bacc.Bacc("TRN2", target_bir_lowering=False, debug=False,
                   num_devices=N_CORES)

    xT_d = nc.declare_dram_parameter("xT", [D, S], F16, isOutput=False)
    wqk_d = nc.declare_dram_parameter("wqk", [NPAIR, P, 2, ND, P], F16,
                                      isOutput=False)
    wvT_d = nc.declare_dram_parameter("wvT", [2, P, ND, FD], F16,
                                      isOutput=False)
    owT_d = nc.declare_dram_parameter("owT", [P, NPAIR, E], BF16,
                                      isOutput=False)
    bqk_d = nc.declare_dram_parameter("bqk", [P, 2 * NPAIR], F32,
                                      isOutput=False)
    out_d = nc.declare_dram_parameter("out", [S, E], F32, isOutput=True)

    with tile.TileContext(nc) as tc:
      # Pools hoisted OUT of the rep loop: consecutive reps share the tile
      # rings, so rep N+1's input DMAs overlap rep N's compute and there is
      # no per-rep pool barrier (steady-state throughput, not latency).
      with (
          tc.tile_pool(name="glob", bufs=1) as glob,
          tc.tile_pool(name="wpool", bufs=1) as wpool,
          tc.tile_pool(name="pwv", bufs=2) as pwv,
          tc.tile_pool(name="pwqk", bufs=3) as pwqk,
          tc.tile_pool(name="pexp", bufs=28) as pexp,
          tc.tile_pool(name="pnrm", bufs=3) as pnrm,
          tc.tile_pool(name="pout", bufs=2) as pout,
          tc.tile_pool(name="psl", bufs=2, space="PSUM") as psl,
          tc.tile_pool(name="psrc", bufs=2, space="PSUM") as psrc,
          tc.tile_pool(name="psav", bufs=2, space="PSUM") as psav,
      ):
        for _rep in range(reps):
            # ---------------- global tiles + DMAs ----------------
            bqk_sb = glob.tile([P, 2 * NPAIR], F32)
            nc.sync.dma_start(bqk_sb[:], bqk_d[:])

            xT_sb = wpool.tile([P, ND, S], F16)
            qT_sb = wpool.tile([P, NPAIR, S], F16)   # [64p+j, pair, s]
            kT_sb = wpool.tile([P, NPAIR, S], F16)
            v_sb = wpool.tile([P, NT, H, Dh + 1], BF16)
            valsN = wpool.tile([P, NPAIR, S], BF16)  # head-pair packed vals^T
            owT_sb = wpool.tile([P, NPAIR, E], BF16)

            # DMA order tuned so qk0's first matmul starts early
            xT_r = xT_d.rearrange("(dt p) s -> p dt s", p=P)
            wqk_t = {}
            w_t = pwqk.tile([P, 2, ND, P], F16, tag="wqk", name="w_t")
            nc.sync.dma_start(w_t[:], wqk_d[0])
            wqk_t[0] = w_t
            nc.sync.dma_start(xT_sb[:, :, 0:FD], xT_r[:, :, 0:FD])
            w_t = pwqk.tile([P, 2, ND, P], F16, tag="wqk", name="w_t")
            nc.sync.dma_start(w_t[:], wqk_d[1])
            wqk_t[1] = w_t
            nc.sync.dma_start(xT_sb[:, :, FD:S], xT_r[:, :, FD:S])
            wv_c = []
            for c in range(2):
                wv = pwv.tile([P, ND, FD], F16, tag="wv", name="wv")
                nc.sync.dma_start(wv[:], wvT_d[c])
                wv_c.append(wv)
            nc.sync.dma_start(owT_sb[:], owT_d[:])

            # ones column of the augmented v + ones stationary for the K=1
            # reciprocal-broadcast matmul (memset can't write f32r/bf16)
            ones_t = glob.tile([P, 1], F32)
            nc.vector.memset(ones_t[:], 1.0)
            nc.vector.tensor_copy(
                out=v_sb[:, :, :, Dh:Dh + 1],
                in_=ones_t[:, None, None, :].to_broadcast((P, NT, H, 1)))
            oc_f = glob.tile([1, P], F32)
            nc.vector.memset(oc_f[:], 1.0)
            oc = glob.tile([1, P], F32R)
            nc.vector.tensor_copy(out=oc[:], in_=oc_f[:])

            # Projections run as interleaved accumulation chains (c0/c1 bank
            # chains alternate, stationary shared) — in-chain matmuls hide
            # their LDWEIGHTS (HW-measured ~242ns/MM vs 417ns for isolated
            # start+stop singles). K-splitting would double the MM count for
            # no stream-time gain, so contractions stay K=128.
            def qk_proj(t):
                """q/k projections for head pair t (own psum ring so they
                prefetch during the previous pair's exp chain)."""
                w_t = wqk_t.pop(t, None)
                if w_t is None:
                    w_t = pwqk.tile([P, 2, ND, P], F16, tag="wqk",
                                    name="w_t")
                    nc.sync.dma_start(w_t[:], wqk_d[t])
                for qk in range(2):
                    dest = qT_sb if qk == 0 else kT_sb
                    ps_c = [psrc.tile([P, FD], F32, tag="rcp", name="ps_c")
                            for _ in range(2)]
                    for dt in range(ND):
                        for c in range(2):
                            nc.tensor.matmul(
                                ps_c[c][:],
                                w_t[:, qk, dt, :],
                                xT_sb[:, dt, FD * c:FD * (c + 1)],
                                start=(dt == 0), stop=(dt == ND - 1))
                    for c in range(2):
                        nc.vector.tensor_scalar(
                            out=dest[:, t, FD * c:FD * (c + 1)],
                            in0=ps_c[c][:],
                            scalar1=bqk_sb[:,
                                           NPAIR * qk + t:NPAIR * qk + t + 1],
                            scalar2=None,
                            op0=mybir.AluOpType.add)

            qk_proj(0)

            # v projection: one [128,1024] group per s-tile (both c halves)
            for st in range(NT):
                ps = psl.tile([P, S], F32, tag="pl", name="ps")
                for dt in range(ND):
                    for c in range(2):
                        nc.tensor.matmul(
                            ps[:, FD * c:FD * (c + 1)],
                            xT_sb[:, dt, P * st:P * (st + 1)],
                            wv_c[c][:, dt, :],
                            start=(dt == 0), stop=(dt == ND - 1))
                nc.vector.tensor_copy(
                    out=v_sb[:, st, :, 0:Dh],
                    in_=ps[:].rearrange("p (c h e) -> p (c h) e", c=2, h=8))

            # ---------------- fused attention pair loop ----------------
            for t in range(NPAIR):
                # logits + exp, st-major. The two heads of the pair live on
                # PE row-groups 0-1 (partitions 0:64) and 2-3 (64:128); with
                # p-interleaved issue the two K=64 matmuls run CONCURRENTLY
                # (tile_position auto-derives from the operand base
                # partition), doubling logits throughput vs serial issue.
                ex_t = [[None, None] for _ in range(NT)]
                for st in range(NT):
                    pl_p = [psl.tile([P, S], F32, tag="pl", name="pl")
                            for _ in range(2)]
                    for c in range(2):
                        for p in range(2):
                            b0 = Dh * p
                            nc.tensor.matmul(
                                pl_p[p][:, FD * c:FD * (c + 1)],
                                kT_sb[b0:b0 + Dh, t, P * st:P * (st + 1)],
                                qT_sb[b0:b0 + Dh, t, FD * c:FD * (c + 1)],
                                start=True, stop=True)
                    for p in range(2):
                        ex = pexp.tile([P, S], BF16, tag="ex", name="ex")
                        nc.scalar.activation(ex[:], pl_p[p][:], EXP)
                        ex_t[st][p] = ex

                # next pair's q/k projections fill PE gaps in the exp chain
                if t + 1 < NPAIR:
                    qk_proj(t + 1)

                # attn@v, query-half (c) outer so pav fits 2 banks
                for c in range(2):
                    pav = [psav.tile([Dh + 1, FD], F32, tag="pav",
                                     name="pav") for _ in range(2)]
                    for st in range(NT):
                        for p in range(2):
                            nc.tensor.matmul(
                                pav[p][:],
                                v_sb[:, st, 2 * t + p, :],
                                ex_t[st][p][:, FD * c:FD * (c + 1)],
                                start=(st == 0), stop=(st == NT - 1))
                    cs = slice(FD * c, FD * (c + 1))
                    for p in range(2):
                        # fast PSUM evac (frees pav), then normalize: DVE
                        # reciprocal of the row-64 denominator, K=1 matmul
                        # partition-broadcast, multiply into bf16 valsN
                        vU = pnrm.tile([Dh + 1, FD], F32, tag="vU",
                                       name="vU")
                        nc.vector.tensor_copy(out=vU[:], in_=pav[p][:])
                        rc = pnrm.tile([1, FD], F32R, tag="rc", name="rc")
                        # f32r is bit-identical fp32 (PE addressing tag)
                        with nc.allow_low_precision(reason="f32r==f32"):
                            nc.vector.reciprocal(rc[:], vU[Dh:Dh + 1, :])
                        rcp = psrc.tile([P, FD], F32, tag="rcp", name="rcp")
                        nc.tensor.matmul(rcp[:], oc[:], rc[:],
                                         start=True, stop=True)
                        if p == 0:
                            nc.vector.tensor_mul(
                                valsN[0:Dh, t, cs], vU[0:Dh, :],
                                rcp[0:Dh, :])
                        else:
                            tmp = pnrm.tile([Dh, FD], BF16, tag="vtmp",
                                            name="tmp")
                            nc.vector.tensor_mul(
                                tmp[:], vU[0:Dh, :], rcp[0:Dh, :])
                            nc.sync.dma_start(valsN[Dh:P, t, cs], tmp[:])

            # ---------------- output projection ----------------
            for m in range(NT):
                ps = psl.tile([P, S], F32, tag="pl", name="ps")
                for tt in range(NPAIR):
                    for c in range(2):
                        nc.tensor.matmul(
                            ps[:, FD * c:FD * (c + 1)],
                            valsN[:, tt, P * m:P * (m + 1)],
                            owT_sb[:, tt, FD * c:FD * (c + 1)],
                            start=(tt == 0), stop=(tt == NPAIR - 1))
                ot = pout.tile([P, S], F32, tag="ot", name="ot")
                nc.scalar.copy(ot[:], ps[:])
                nc.sync.dma_start(out_d[P * m:P * (m + 1), :], ot[:])

    nc.compile()
    return nc


_NC_CACHE = {}


def get_nc():
    if "nc" not in _NC_CACHE:
        _NC_CACHE["nc"] = build_nc()
    return _NC_CACHE["nc"]


def prepare_inputs(x, qkv_w, qkv_b, o_w, o_b):
    """Host-side layout packing. Returns (in_maps, correction)."""
    x = np.asarray(x, dtype=np.float32)
    qkv_w = np.asarray(qkv_w, dtype=np.float32)
    qkv_b = np.asarray(qkv_b, dtype=np.float32)
    o_w = np.asarray(o_w, dtype=np.float32)
    o_b = np.asarray(o_b, dtype=np.float32)

    w3 = qkv_w.reshape(H, 3 * Dh, D)
    wq = w3[:, 0:Dh, :].reshape(E, D)        # row 64h+j = q_j of head h
    wk = w3[:, Dh:2 * Dh, :].reshape(E, D)
    wv = w3[:, 2 * Dh:, :].reshape(E, D)

    wqk = np.concatenate([wq, wk], axis=0)   # [2048, 1024]
    wqkT = np.ascontiguousarray(wqk.T)       # [D, 2048]
    # [2*NPAIR, P, ND, P] with tile index t (q) / t+NPAIR (k) ...
    wqk_tiles = wqkT.reshape(ND, P, 2 * NPAIR, P).transpose(2, 1, 0, 3)
    # ... re-packed so pair t = [q_t, k_t] arrives in one DMA
    wqk_pair = np.stack(
        [np.stack([wqk_tiles[t], wqk_tiles[t + NPAIR]], axis=1)
         for t in range(NPAIR)], axis=0)     # [NPAIR, P, 2, ND, P]
    wqk_pair = np.ascontiguousarray(wqk_pair).astype(np.float16)

    wvT = np.ascontiguousarray(wv.T)         # [D, E]
    wvT_tiled = np.ascontiguousarray(
        wvT.reshape(ND, P, 2, FD).transpose(2, 1, 0, 3)).astype(np.float16)

    owT = np.ascontiguousarray(o_w.T)        # [E, E]; row e = 128t + r
    owT_pair = np.ascontiguousarray(
        owT.reshape(NPAIR, P, E).transpose(1, 0, 2)).astype(ml_dtypes.bfloat16)

    b3 = qkv_b.reshape(H, 3 * Dh)
    bq, bk, bv = b3[:, 0:Dh], b3[:, Dh:2 * Dh], b3[:, 2 * Dh:]
    cols = [np.concatenate([bq[2 * t], bq[2 * t + 1]]) for t in range(NPAIR)]
    cols += [np.concatenate([bk[2 * t], bk[2 * t + 1]]) for t in range(NPAIR)]
    bqk = np.ascontiguousarray(np.stack(cols, axis=1))  # [128, 16]

    correction = bv.reshape(E) @ o_w.T + o_b            # [E]

    in_maps = []
    for b in range(B):
        in_maps.append({
            "xT": np.ascontiguousarray(x[b].T).astype(np.float16),
            "wqk": wqk_pair,
            "wvT": wvT_tiled,
            "owT": owT_pair,
            "bqk": bqk,
        })
    return in_maps, correction


def kernel(x, qkv_w, qkv_b, o_w, o_b):
    nc = get_nc()
    in_maps, correction = prepare_inputs(x, qkv_w, qkv_b, o_w, o_b)
    res = run_bass_kernel_spmd(nc, in_maps, list(range(N_CORES)))
    out = np.stack([res.results[b]["out"] for b in range(B)], axis=0)
    out = out + correction[None, None, :]
    return out.astype(np.float32)



# revision 15
# speedup vs baseline: 1.0698x; 1.0495x over previous
"""MultiHeadAttention Trainium2 kernel (8 NeuronCores, data-parallel over batch).

Problem: B=8, S=1024, D=1024, E=1024, H=16 heads, Dh=64.
  qkv = x @ qkv_w.T + qkv_b ; per head: softmax(q k^T) @ v ; out = vals @ o_w.T + o_b
  (softmax on UNSCALED logits, faithful to the reference.)

Strategy (v12 — interleaved emission pipeline)
----------------------------------------------
- Data-parallel: core b processes batch element b completely. No collectives.
- Mixed precision validated vs the fp64 reference (rel err ~3.2e-3, gate 2e-2):
  fp16 for x / qkv(q,k) weights / q / k (logits path needs the mantissa),
  bf16 for v / exp(logits) / normalized vals / o_w (exp needs bf16 range).
  fp8 was analyzed and rejected: softmax on UNSCALED logits (std ~4)
  amplifies q/k quantization exp-fold, and e4m3 on v/ex/o alone costs
  2-5% rel err vs the 2e-2 gate.
- Engine queues are strict FIFO, so the key lever (HW-measured via
  microbenchmarks + npairs ablation) is EMISSION-ORDER interleaving:
  every PE-heavy chain (qk-proj, v-proj, attn@v+normalize) is a generator
  yielding ~1-2us chunks, drained as "filler" between the s-tiles of the
  ACT-bound logits/exp chain. attn@v(t-1) and qk_proj(t+1) fill pair t;
  v-proj fills pair 0. This collapsed ~46us/pair (sequential phases) to
  ~max(ACT 18.4us, PE ~21us).
- Logits issue p-interleaved: the two K=64 heads run on PE row-groups 0-1 /
  2-3 concurrently (auto tile_position; explicit tile_position args crash
  the runtime). HW: 160ns/MM vs 417ns for isolated full-K singles.
  Projections stay K=128 accumulation chains (in-chain LDWEIGHTS hides:
  242ns/MM; K-splitting would double MM count for no stream-time gain).
- Pools live OUTSIDE the rep loop: rep N+1's input DMAs overlap rep N's
  compute (steady-state throughput; reps>1 used only for timing).
- PSUM 8 banks: pl ring 2x[128,1024] (logits + o-proj), qk-proj ring
  2x[128,512], pav ring 2x[128,512] shared by v-proj chunks / attn@v
  accumulators / reciprocal broadcasts.
- attn@v rides a ones column producing the softmax denominator in row 64;
  evac+reciprocal are DVE-only chunks emitted one chunk before the PE
  broadcast that consumes them, so the PE never head-blocks on DVE.
- o_b and the v-bias contribution are folded in on the host (softmax rows
  sum to 1).
"""

import numpy as np
import ml_dtypes

import concourse.bass as bass
import concourse.tile as tile
from concourse import bacc, mybir
from concourse.bass_utils import run_bass_kernel_spmd

F32 = mybir.dt.float32
F32R = mybir.dt.float32r
F16 = mybir.dt.float16
BF16 = mybir.dt.bfloat16
EXP = mybir.ActivationFunctionType.Exp

B, S, D, E, H, Dh = 8, 1024, 1024, 1024, 16, 64
P = 128          # partitions
NT = S // P      # 8 s-tiles
ND = D // P      # 8 d-tiles
NPAIR = H // 2   # 8 head-pair tiles
FD = 512         # matmul moving free dim

N_CORES = 8


def build_nc(reps: int = 1, npairs: int = NPAIR):
    nc = bacc.Bacc("TRN2", target_bir_lowering=False, debug=False,
                   num_devices=N_CORES)

    xT_d = nc.declare_dram_parameter("xT", [D, S], F16, isOutput=False)
    wqk_d = nc.declare_dram_parameter("wqk", [NPAIR, P, 2, ND, P], F16,
                                      isOutput=False)
    wvT_d = nc.declare_dram_parameter("wvT", [2, P, ND, FD], F16,
                                      isOutput=False)
    owT_d = nc.declare_dram_parameter("owT", [P, NPAIR, E], BF16,
                                      isOutput=False)
    bqk_d = nc.declare_dram_parameter("bqk", [P, 2 * NPAIR], F32,
                                      isOutput=False)
    out_d = nc.declare_dram_parameter("out", [S, E], F32, isOutput=True)

    with tile.TileContext(nc) as tc:
      # Pools hoisted OUT of the rep loop: consecutive reps share the tile
      # rings, so rep N+1's input DMAs overlap rep N's compute and there is
      # no per-rep pool barrier (steady-state throughput, not latency).
      with (
          tc.tile_pool(name="glob", bufs=1) as glob,
          tc.tile_pool(name="wpool", bufs=1) as wpool,
          tc.tile_pool(name="pwv", bufs=2) as pwv,
          tc.tile_pool(name="pwqk", bufs=3) as pwqk,
          tc.tile_pool(name="pexp", bufs=30) as pexp,
          tc.tile_pool(name="pnrm", bufs=3) as pnrm,
          tc.tile_pool(name="pout", bufs=2) as pout,
          tc.tile_pool(name="psl", bufs=2, space="PSUM") as psl,
          tc.tile_pool(name="psrc", bufs=2, space="PSUM") as psrc,
          tc.tile_pool(name="psav", bufs=2, space="PSUM") as psav,
      ):
        for _rep in range(reps):
            # ---------------- global tiles + DMAs ----------------
            bqk_sb = glob.tile([P, 2 * NPAIR], F32)
            nc.sync.dma_start(bqk_sb[:], bqk_d[:])

            xT_sb = wpool.tile([P, ND, S], F16)
            qT_sb = wpool.tile([P, NPAIR, S], F16)   # [64p+j, pair, s]
            kT_sb = wpool.tile([P, NPAIR, S], F16)
            v_sb = wpool.tile([P, NT, H, Dh + 1], BF16)
            valsN = wpool.tile([P, NPAIR, S], BF16)  # head-pair packed vals^T
            owT_sb = wpool.tile([P, NPAIR, E], BF16)

            # DMA order tuned so qk0's first matmul starts early
            xT_r = xT_d.rearrange("(dt p) s -> p dt s", p=P)
            wqk_t = {}
            w_t = pwqk.tile([P, 2, ND, P], F16, tag="wqk", name="w_t")
            nc.sync.dma_start(w_t[:], wqk_d[0])
            wqk_t[0] = w_t
            nc.sync.dma_start(xT_sb[:, :, 0:FD], xT_r[:, :, 0:FD])
            w_t = pwqk.tile([P, 2, ND, P], F16, tag="wqk", name="w_t")
            nc.sync.dma_start(w_t[:], wqk_d[1])
            wqk_t[1] = w_t
            nc.sync.dma_start(xT_sb[:, :, FD:S], xT_r[:, :, FD:S])
            wv_c = []
            for c in range(2):
                wv = pwv.tile([P, ND, FD], F16, tag="wv", name="wv")
                nc.sync.dma_start(wv[:], wvT_d[c])
                wv_c.append(wv)
            nc.sync.dma_start(owT_sb[:], owT_d[:])

            # ones column of the augmented v + ones stationary for the K=1
            # reciprocal-broadcast matmul (memset can't write f32r/bf16)
            ones_t = glob.tile([P, 1], F32)
            nc.vector.memset(ones_t[:], 1.0)
            nc.vector.tensor_copy(
                out=v_sb[:, :, :, Dh:Dh + 1],
                in_=ones_t[:, None, None, :].to_broadcast((P, NT, H, 1)))
            oc_f = glob.tile([1, P], F32)
            nc.vector.memset(oc_f[:], 1.0)
            oc = glob.tile([1, P], F32R)
            nc.vector.tensor_copy(out=oc[:], in_=oc_f[:])

            # ---- generator-based emission pipeline -------------------
            # The engines execute their queues in strict FIFO order, so a
            # blocked instruction at the head idles the whole engine. The
            # logits/exp chain is ACT-bound (~2.3us per s-tile) while the
            # projection / attn@v chains are PE-bound; emitting them in
            # separate phases serialized the two (HW: ~46us per pair vs
            # ~21us of engine work). Instead, every PE-heavy block below is
            # a generator that yields at ~1-2us chunk boundaries, and the
            # logits loop drains a filler queue between s-tiles so the PE
            # queue always holds ready work while ACT churns through exp:
            #   pair t fillers = attn@v(t-1) + normalize, qk_proj(t+1);
            #   pair 0 fillers = v-proj, qk_proj(1).
            # In-chain matmuls hide their LDWEIGHTS (HW: ~242ns/MM vs 417ns
            # for isolated singles), so contractions stay K=128 chains.

            def qk_proj_gen(t):
                """q/k projections for head pair t; 4 chunks."""
                w_t = wqk_t.pop(t, None)
                if w_t is None:
                    w_t = pwqk.tile([P, 2, ND, P], F16, tag="wqk",
                                    name="w_t")
                    nc.sync.dma_start(w_t[:], wqk_d[t])
                for qk in range(2):
                    dest = qT_sb if qk == 0 else kT_sb
                    ps_c = [psrc.tile([P, FD], F32, tag="qkp", name="ps_c")
                            for _ in range(2)]
                    for dh in range(2):
                        for dt in range(4 * dh, 4 * dh + 4):
                            for c in range(2):
                                nc.tensor.matmul(
                                    ps_c[c][:],
                                    w_t[:, qk, dt, :],
                                    xT_sb[:, dt, FD * c:FD * (c + 1)],
                                    start=(dt == 0), stop=(dt == ND - 1))
                        if dh == 1:
                            for c in range(2):
                                nc.vector.tensor_scalar(
                                    out=dest[:, t, FD * c:FD * (c + 1)],
                                    in0=ps_c[c][:],
                                    scalar1=bqk_sb[:, NPAIR * qk + t:
                                                   NPAIR * qk + t + 1],
                                    scalar2=None,
                                    op0=mybir.AluOpType.add)
                        yield

            def vp_gen():
                """v projection; 8 chunks (one per s-tile)."""
                for st in range(NT):
                    pv = [psav.tile([P, FD], F32, tag="pav", name="pv")
                          for _ in range(2)]
                    for dt in range(ND):
                        for c in range(2):
                            nc.tensor.matmul(
                                pv[c][:],
                                xT_sb[:, dt, P * st:P * (st + 1)],
                                wv_c[c][:, dt, :],
                                start=(dt == 0), stop=(dt == ND - 1))
                    for c in range(2):
                        nc.vector.tensor_copy(
                            out=v_sb[:, st, 8 * c:8 * (c + 1), 0:Dh],
                            in_=pv[c][:].rearrange("p (h e) -> p h e", h=8))
                    yield

            def av_gen(t, ex_t):
                """attn@v + normalize for pair t; 8 chunks. The DVE-only
                evac/reciprocal chunk is emitted one chunk before the PE
                broadcast that consumes it, so the PE never head-blocks on
                the DVE chain."""
                for c in range(2):
                    pav = [psav.tile([Dh + 1, FD], F32, tag="pav",
                                     name="pav") for _ in range(2)]
                    for sh in range(2):
                        for st in range(4 * sh, 4 * sh + 4):
                            for p in range(2):
                                nc.tensor.matmul(
                                    pav[p][:],
                                    v_sb[:, st, 2 * t + p, :],
                                    ex_t[st][p][:, FD * c:FD * (c + 1)],
                                    start=(st == 0), stop=(st == NT - 1))
                        yield
                    # DVE-only: fast PSUM evac (frees pav) + reciprocal of
                    # the row-64 softmax denominator
                    vU, rc = [], []
                    for p in range(2):
                        v_ = pnrm.tile([Dh + 1, FD], F32, tag="vU", bufs=2,
                                       name="v_")
                        nc.vector.tensor_copy(out=v_[:], in_=pav[p][:])
                        r_ = pnrm.tile([1, FD], F32R, tag="rc", bufs=2, name="r_")
                        # f32r is bit-identical fp32 (PE addressing tag)
                        with nc.allow_low_precision(reason="f32r==f32"):
                            nc.vector.reciprocal(r_[:], v_[Dh:Dh + 1, :])
                        vU.append(v_)
                        rc.append(r_)
                    yield
                    # K=1 matmul partition-broadcast of 1/denominator, then
                    # multiply into bf16 valsN (p=1 bounces via DMA to reach
                    # partitions 64:128)
                    cs = slice(FD * c, FD * (c + 1))
                    for p in range(2):
                        rcp = psav.tile([P, FD], F32, tag="pav", name="rcp")
                        nc.tensor.matmul(rcp[:], oc[:], rc[p][:],
                                         start=True, stop=True)
                        if p == 0:
                            nc.vector.tensor_mul(
                                valsN[0:Dh, t, cs], vU[p][0:Dh, :],
                                rcp[0:Dh, :])
                        else:
                            tmp = pnrm.tile([Dh, FD], BF16, tag="vtmp", bufs=2,
                                            name="tmp")
                            nc.vector.tensor_mul(
                                tmp[:], vU[p][0:Dh, :], rcp[0:Dh, :])
                            nc.sync.dma_start(valsN[Dh:P, t, cs], tmp[:])
                    yield

            fillers = []

            def drive(n):
                while n > 0 and fillers:
                    try:
                        next(fillers[0])
                        n -= 1
                    except StopIteration:
                        fillers.pop(0)

            if npairs > 0:
                for _ in qk_proj_gen(0):
                    pass
            fillers.append(vp_gen())
            if npairs > 1:
                fillers.append(qk_proj_gen(1))

            # ---------------- fused attention pair loop ----------------
            for t in range(npairs):
                # logits + exp, st-major. The two heads of the pair live on
                # PE row-groups 0-1 (partitions 0:64) and 2-3 (64:128); with
                # p-interleaved issue the two K=64 matmuls run CONCURRENTLY
                # (tile_position auto-derives from the operand base
                # partition), doubling logits throughput vs serial issue.
                ex_t = [[None, None] for _ in range(NT)]
                for st in range(NT):
                    pl_p = [psl.tile([P, S], F32, tag="pl", name="pl")
                            for _ in range(2)]
                    for c in range(2):
                        for p in range(2):
                            b0 = Dh * p
                            nc.tensor.matmul(
                                pl_p[p][:, FD * c:FD * (c + 1)],
                                kT_sb[b0:b0 + Dh, t, P * st:P * (st + 1)],
                                qT_sb[b0:b0 + Dh, t, FD * c:FD * (c + 1)],
                                start=True, stop=True)
                    for p in range(2):
                        ex = pexp.tile([P, S], BF16, tag="ex", name="ex")
                        nc.scalar.activation(ex[:], pl_p[p][:], EXP)
                        ex_t[st][p] = ex
                    drive(2)
                fillers.append(av_gen(t, ex_t))
                if t + 2 < npairs:
                    fillers.append(qk_proj_gen(t + 2))
                drive(2)
            drive(10 ** 9)

            # ---------------- output projection ----------------
            for m in range(NT):
                ps = psl.tile([P, S], F32, tag="pl", name="ps")
                for tt in range(NPAIR):
                    for c in range(2):
                        nc.tensor.matmul(
                            ps[:, FD * c:FD * (c + 1)],
                            valsN[:, tt, P * m:P * (m + 1)],
                            owT_sb[:, tt, FD * c:FD * (c + 1)],
                            start=(tt == 0), stop=(tt == NPAIR - 1))
                ot = pout.tile([P, S], F32, tag="ot", name="ot")
                nc.scalar.copy(ot[:], ps[:])
                nc.sync.dma_start(out_d[P * m:P * (m + 1), :], ot[:])

    nc.compile()
    return nc


_NC_CACHE = {}


def get_nc():
    if "nc" not in _NC_CACHE:
        _NC_CACHE["nc"] = build_nc()
    return _NC_CACHE["nc"]


def prepare_inputs(x, qkv_w, qkv_b, o_w, o_b):
    """Host-side layout packing. Returns (in_maps, correction)."""
    x = np.asarray(x, dtype=np.float32)
    qkv_w = np.asarray(qkv_w, dtype=np.float32)
    qkv_b = np.asarray(qkv_b, dtype=np.float32)
    o_w = np.asarray(o_w, dtype=np.float32)
    o_b = np.asarray(o_b, dtype=np.float32)

    w3 = qkv_w.reshape(H, 3 * Dh, D)
    wq = w3[:, 0:Dh, :].reshape(E, D)        # row 64h+j = q_j of head h
    wk = w3[:, Dh:2 * Dh, :].reshape(E, D)
    wv = w3[:, 2 * Dh:, :].reshape(E, D)

    wqk = np.concatenate([wq, wk], axis=0)   # [2048, 1024]
    wqkT = np.ascontiguousarray(wqk.T)       # [D, 2048]
    # [2*NPAIR, P, ND, P] with tile index t (q) / t+NPAIR (k) ...
    wqk_tiles = wqkT.reshape(ND, P, 2 * NPAIR, P).transpose(2, 1, 0, 3)
    # ... re-packed so pair t = [q_t, k_t] arrives in one DMA
    wqk_pair = np.stack(
        [np.stack([wqk_tiles[t], wqk_tiles[t + NPAIR]], axis=1)
         for t in range(NPAIR)], axis=0)     # [NPAIR, P, 2, ND, P]
    wqk_pair = np.ascontiguousarray(wqk_pair).astype(np.float16)

    wvT = np.ascontiguousarray(wv.T)         # [D, E]
    wvT_tiled = np.ascontiguousarray(
        wvT.reshape(ND, P, 2, FD).transpose(2, 1, 0, 3)).astype(np.float16)

    owT = np.ascontiguousarray(o_w.T)        # [E, E]; row e = 128t + r
    owT_pair = np.ascontiguousarray(
        owT.reshape(NPAIR, P, E).transpose(1, 0, 2)).astype(ml_dtypes.bfloat16)

    b3 = qkv_b.reshape(H, 3 * Dh)
    bq, bk, bv = b3[:, 0:Dh], b3[:, Dh:2 * Dh], b3[:, 2 * Dh:]
    cols = [np.concatenate([bq[2 * t], bq[2 * t + 1]]) for t in range(NPAIR)]
    cols += [np.concatenate([bk[2 * t], bk[2 * t + 1]]) for t in range(NPAIR)]
    bqk = np.ascontiguousarray(np.stack(cols, axis=1))  # [128, 16]

    correction = bv.reshape(E) @ o_w.T + o_b            # [E]

    in_maps = []
    for b in range(B):
        in_maps.append({
            "xT": np.ascontiguousarray(x[b].T).astype(np.float16),
            "wqk": wqk_pair,
            "wvT": wvT_tiled,
            "owT": owT_pair,
            "bqk": bqk,
        })
    return in_maps, correction


def kernel(x, qkv_w, qkv_b, o_w, o_b):
    nc = get_nc()
    in_maps, correction = prepare_inputs(x, qkv_w, qkv_b, o_w, o_b)
    res = run_bass_kernel_spmd(nc, in_maps, list(range(N_CORES)))
    out = np.stack([res.results[b]["out"] for b in range(B)], axis=0)
    out = out + correction[None, None, :]
    return out.astype(np.float32)

